# revision 7
# baseline (speedup 1.0000x reference)
"""Trainium2 Bass kernel for a custom GRU cell.

    x_h   = concat([inputs, h_prev], -1)            # [B, D+U]
    z     = sigmoid(x_h @ Wz)                       # [B, U]
    r     = sigmoid(x_h @ Wr)                       # [B, U]
    h_hat = tanh(concat([inputs, r * h_prev]) @ Wh) # [B, U]
    out   = z * h_prev + (1 - z) * h_hat
          = h_hat + z * (h_prev - h_hat)

Data-parallel over 8 NeuronCores: batch sharded, weights replicated.

Staging: inputs are cast to bf16 and laid out feature-major (x.T, h.T)
on the host; the output is produced transposed [U, B_c] and transposed
back on the host. All matmul/activation/elementwise work runs on
device.

Per-core structure (B_c = 2048 cols = 4 groups of 512), ALL GATES
TRANSPOSED, weight-stationary:

    ps[u,g] += W*[k][:, 128u:].T @ xh[k][:, 512g:]

Each stationary weight tile W[k][:,u] serves 4 consecutive matmuls
(batch groups g=0..3), so LDWEIGHTS is amortized 4x — the v2 profile
showed a 116 ns LDWEIGHTS fully serialized with every 259 ns matmul.
r*h_prev is computed transposed (rT * hT, hT = xh[4+u]) and feeds the
gate-h matmuls as the MOVING operand; the combine
out = hh + z*(h - hh) happens in transposed space on VectorE.
PSUM: each u-chain holds 4 banks over its k-loop; two chains overlap.
"""

import sys

for _p in ("/opt/trn_rl_repo", "/root/.axon_site/_ro/trn_rl_repo"):
    if _p not in sys.path:
        sys.path.append(_p)

import numpy as np

B, D, U = 16384, 512, 512
K = D + U
N_CORES = 8
BC = B // N_CORES          # rows per core (2048)
GW = 512                   # batch-group width
G = BC // GW               # groups per core (4)
KC = K // 128              # contraction chunks (8)


def build_gru_tile_kernel(tc, d_xT, d_hT, d_wz, d_wr, d_wh, d_outT):
    """Emit the GRU cell body into TileContext `tc`."""
    import contextlib

    from concourse import mybir

    f32 = mybir.dt.float32
    bf16 = mybir.dt.bfloat16
    nc = tc.nc
    Sig = mybir.ActivationFunctionType.Sigmoid
    Tanh = mybir.ActivationFunctionType.Tanh

    est = contextlib.ExitStack()
    wpool = est.enter_context(tc.tile_pool(name="w", bufs=1))
    xhp = est.enter_context(tc.tile_pool(name="xh", bufs=1))
    rtp = est.enter_context(tc.tile_pool(name="rt", bufs=4))
    rhp = est.enter_context(tc.tile_pool(name="rh", bufs=1))
    zpool = est.enter_context(tc.tile_pool(name="z", bufs=1))
    hhp = est.enter_context(tc.tile_pool(name="hh", bufs=4))
    tmpp = est.enter_context(tc.tile_pool(name="tmp", bufs=8))
    outp = est.enter_context(tc.tile_pool(name="out", bufs=8))
    psg = est.enter_context(tc.tile_pool(name="psg", bufs=8, space="PSUM"))

    # ---- DMA schedule ----
    # xh chunk k + Wr chunk k stream in together (640 KB per k step,
    # ~1.8 us) and unlock the next accumulation step of the two r-gate
    # u-chains in flight (8 matmuls, ~1.7 us) — the PE trails DMA by
    # almost nothing during the ramp. Wz/Wh follow; they are consumed
    # a full gate-phase (~25 us) later.
    xh = [None] * KC
    w_sb = {}
    for name in ("wr", "wz", "wh"):
        w_sb[name] = wpool.tile([128, KC, U], bf16, tag=name, name=name)

    def load_xh(k):
        src = d_xT if k < 4 else d_hT
        kk = k % 4
        t = xhp.tile([128, BC], bf16, tag=f"xh_{k}", name=f"xh_{k}")
        nc.sync.dma_start(t[:], src[128 * kk:128 * (kk + 1), :])
        xh[k] = t

    def load_w(name, dram, k):
        nc.sync.dma_start(w_sb[name][:, k, :],
                          dram[128 * k:128 * (k + 1), :])

    for k in range(KC):
        load_xh(k)
        load_w("wr", d_wr, k)
    for k in range(KC):
        load_w("wz", d_wz, k)
    for k in range(KC):
        load_w("wh", d_wh, k)

    # rh[u] and z[u] accumulate full-width [128, 2048] results per
    # u-chunk; slices are written per (u, g) as chains retire.
    rh = [rhp.tile([128, BC], bf16, tag=f"rh_{u}", name=f"rh_{u}")
          for u in range(4)]
    zt = [zpool.tile([128, BC], bf16, tag=f"z_{u}", name=f"z_{u}")
          for u in range(4)]

    import bass_rust
    NOSYNC = bass_rust.DependencyInfo.NO_SYNC_ONLY
    blocks = nc.m.functions[0].blocks

    def raw_last():
        return blocks[-1].instructions[-1]

    # Explicit LDWEIGHTS shared by the 4 batch-group matmuls. The PE
    # stream is kept in a single total order (each LDW nosync-depends on
    # the previous matmul, each matmul on its LDW) so the scheduler can
    # never slip another chain's weight load between a load and its
    # matmuls.
    prev_mm = [None]

    def chain(wname, u, kslice, moving, ps=None, first=True, last=True):
        """One u-chunk chain: ps[g] += W[k][:,u].T @ moving[k][:,g] —
        k-outer, g-inner so each stationary tile serves 4 matmuls."""
        w = w_sb[wname]
        nk = len(kslice)
        if ps is None:
            ps = [psg.tile([128, GW], f32, tag="psg",
                           name=f"ps_{wname}_{u}_{g}") for g in range(4)]
        for i, k in enumerate(kslice):
            lhsT = w[:, k, 128 * u:128 * (u + 1)]
            nc.tensor.ldweights(lhsT)
            ldw = raw_last()
            if prev_mm[0] is not None:
                ldw.add_dependency(prev_mm[0], NOSYNC)
            for g in range(4):
                nc.tensor.matmul(ps[g][:], lhsT,
                                 moving[i][:, GW * g:GW * (g + 1)],
                                 start=(first and i == 0),
                                 stop=(last and i == nk - 1))
                mm = raw_last()
                mm.ldweights = False
                mm.add_dependency(ldw.name, NOSYNC)
                prev_mm[0] = mm.name
        return ps

    # ---- gate r (transposed) ----
    for u in range(4):
        ps = chain("wr", u, range(KC), xh)
        for g in range(4):
            c0 = GW * g
            rT = rtp.tile([128, GW], bf16, tag="rt", name=f"rt_{u}_{g}")
            nc.scalar.activation(rT[:], ps[g][:], Sig)
            nc.vector.tensor_mul(rh[u][:, c0:c0 + GW], rT[:],
                                 xh[4 + u][:, c0:c0 + GW])

    # ---- gate z (transposed) ----
    for u in range(4):
        ps = chain("wz", u, range(KC), xh)
        for g in range(4):
            nc.scalar.activation(zt[u][:, GW * g:GW * (g + 1)],
                                 ps[g][:], Sig)

    # ---- gate h (transposed) + combine + store ----
    for u in range(4):
        ps = chain("wh", u, range(4), xh, last=False)
        ps = chain("wh", u, range(4, KC), rh, ps=ps, first=False)
        for g in range(4):
            c0 = GW * g
            hh = hhp.tile([128, GW], f32, tag="hh", name=f"hh_{u}_{g}")
            nc.scalar.activation(hh[:], ps[g][:], Tanh)
            # out = hh + z * (h_prev - hh), all transposed
            dt = tmpp.tile([128, GW], f32, tag="tmp", name=f"d_{u}_{g}")
            nc.vector.tensor_sub(dt[:], xh[4 + u][:, c0:c0 + GW], hh[:])
            pt = tmpp.tile([128, GW], f32, tag="tmp2", name=f"p_{u}_{g}")
            nc.vector.tensor_mul(pt[:], zt[u][:, c0:c0 + GW], dt[:])
            o = outp.tile([128, GW], bf16, tag="out", name=f"o_{u}_{g}")
            nc.vector.tensor_add(o[:], hh[:], pt[:])
            nc.sync.dma_start(
                d_outT[128 * u:128 * (u + 1), c0:c0 + GW], o[:])

    est.close()


_NC_CACHE = {}


def _build():
    if "nc" in _NC_CACHE:
        return _NC_CACHE["nc"]
    import concourse.tile as tile
    from concourse import bacc, mybir

    bf16 = mybir.dt.bfloat16
    nc = bacc.Bacc("TRN2", target_bir_lowering=False, debug=False)
    d_xT = nc.dram_tensor("xT", [D, BC], bf16, kind="ExternalInput").ap()
    d_hT = nc.dram_tensor("hT", [U, BC], bf16, kind="ExternalInput").ap()
    d_wz = nc.dram_tensor("Wz", [K, U], bf16, kind="ExternalInput").ap()
    d_wr = nc.dram_tensor("Wr", [K, U], bf16, kind="ExternalInput").ap()
    d_wh = nc.dram_tensor("Wh", [K, U], bf16, kind="ExternalInput").ap()
    d_outT = nc.dram_tensor("outT", [U, BC], bf16, kind="ExternalOutput").ap()

    with tile.TileContext(nc) as tc:
        build_gru_tile_kernel(tc, d_xT, d_hT, d_wz, d_wr, d_wh, d_outT)
    nc.compile()
    _NC_CACHE["nc"] = nc
    return nc


def run_sharded(inputs, h_prev, Wz, Wr, Wh, trace=False):
    import ml_dtypes
    from concourse.bass_utils import run_bass_kernel_spmd

    bf16 = ml_dtypes.bfloat16
    nc = _build()
    inputs = np.asarray(inputs, dtype=np.float32).astype(bf16)
    h_prev = np.asarray(h_prev, dtype=np.float32).astype(bf16)
    Wzq = np.ascontiguousarray(np.asarray(Wz, dtype=np.float32).astype(bf16))
    Wrq = np.ascontiguousarray(np.asarray(Wr, dtype=np.float32).astype(bf16))
    Whq = np.ascontiguousarray(np.asarray(Wh, dtype=np.float32).astype(bf16))
    in_maps = []
    for i in range(N_CORES):
        in_maps.append({
            "xT": np.ascontiguousarray(inputs[i * BC:(i + 1) * BC].T),
            "hT": np.ascontiguousarray(h_prev[i * BC:(i + 1) * BC].T),
            "Wz": Wzq,
            "Wr": Wrq,
            "Wh": Whq,
        })
    res = run_bass_kernel_spmd(
        nc, in_maps, core_ids=list(range(N_CORES)), trace=trace
    )
    out = np.concatenate(
        [np.asarray(res.results[i]["outT"]).T for i in range(N_CORES)], axis=0
    ).astype(np.float32)
    return out, res


def kernel(inputs, h_prev, Wz, Wr, Wh):
    out, _ = run_sharded(inputs, h_prev, Wz, Wr, Wh, trace=False)
    return out


# revision 10
# speedup vs baseline: 1.0370x; 1.0370x over previous
"""Trainium2 Bass kernel for a custom GRU cell.

    x_h   = concat([inputs, h_prev], -1)            # [B, D+U]
    z     = sigmoid(x_h @ Wz)                       # [B, U]
    r     = sigmoid(x_h @ Wr)                       # [B, U]
    h_hat = tanh(concat([inputs, r * h_prev]) @ Wh) # [B, U]
    out   = z * h_prev + (1 - z) * h_hat
          = h_hat + z * (h_prev - h_hat)

Data-parallel over 8 NeuronCores: batch sharded, weights replicated.

Staging: inputs cast to bf16 and laid out feature-major (x.T, h.T) on
the host; output produced transposed [U, B_c] bf16 and transposed back
on the host. All matmul/activation/elementwise work runs on device.

Per-core (B_c = 2048 cols = 4 groups of 512), ALL GATES TRANSPOSED,
weight-stationary with EXPLICIT LDWEIGHTS:

    ps[u,g] += W*[k][:, 128u:].T @ xh[k][:, 512g:]

One InstLdweights per (gate,u,k) serves the 4 batch-group matmuls
(nc.tensor.matmul emits a weight load per matmul otherwise — profiled
at 259 ns/MM vs the 216 ns stream limit this reaches). The PE stream
is a single nosync-ordered chain so no foreign weight load can land
between a load and its matmuls.

Schedule: r-gate u-chains run PAIR-INTERLEAVED k-major (8 PSUM banks,
8 matmuls per arriving k-chunk) so the DMA ramp keeps the PE ~95%
fed; z/h run as sequential u-chains (dense post-ramp, and their
sigmoid/tanh+combine work staggers across the stream). The last
h-chain (u=3) is split into two 2-bank half-chains so its tanh +
combine + store tail shrinks to ~one slice. Combine is all-bf16 on
VectorE (2x DVE rate): out = hh + z*(h - hh).
"""

import sys

for _p in ("/opt/trn_rl_repo", "/root/.axon_site/_ro/trn_rl_repo"):
    if _p not in sys.path:
        sys.path.append(_p)

import numpy as np

B, D, U = 16384, 512, 512
K = D + U
N_CORES = 8
BC = B // N_CORES          # rows per core (2048)
GW = 512                   # batch-group width
G = BC // GW               # groups per core (4)
KC = K // 128              # contraction chunks (8)


def build_gru_tile_kernel(tc, d_xT, d_hT, d_wz, d_wr, d_wh, d_outT):
    """Emit the GRU cell body into TileContext `tc`."""
    import contextlib

    import bass_rust
    from concourse import mybir

    f32 = mybir.dt.float32
    bf16 = mybir.dt.bfloat16
    nc = tc.nc
    Sig = mybir.ActivationFunctionType.Sigmoid
    Tanh = mybir.ActivationFunctionType.Tanh
    NOSYNC = bass_rust.DependencyInfo.NO_SYNC_ONLY

    est = contextlib.ExitStack()
    wpool = est.enter_context(tc.tile_pool(name="w", bufs=1))
    xhp = est.enter_context(tc.tile_pool(name="xh", bufs=1))
    rtp = est.enter_context(tc.tile_pool(name="rt", bufs=4))
    rhp = est.enter_context(tc.tile_pool(name="rh", bufs=1))
    zpool = est.enter_context(tc.tile_pool(name="z", bufs=1))
    hhp = est.enter_context(tc.tile_pool(name="hh", bufs=4))
    tmpp = est.enter_context(tc.tile_pool(name="tmp", bufs=8))
    outp = est.enter_context(tc.tile_pool(name="out", bufs=8))
    psg = est.enter_context(tc.tile_pool(name="psg", bufs=8, space="PSUM"))

    # ---- DMA schedule ----
    # Wr chunk k lands just before xh chunk k (whose 4 column-block DMAs
    # pace the 4 batch-group matmuls of that k step), so the two
    # pair-interleaved r-gate u-chains stream at DMA rate from ~9 us.
    # Wz/Wh follow — they are consumed a gate-phase (~15-40 us) later.
    xh = [None] * KC
    w_sb = {}
    for name in ("wr", "wz", "wh"):
        w_sb[name] = wpool.tile([128, KC, U], bf16, tag=name, name=name)

    def load_w(name, dram, k):
        nc.sync.dma_start(w_sb[name][:, k, :],
                          dram[128 * k:128 * (k + 1), :])

    for k in range(KC):
        src = d_xT if k < 4 else d_hT
        kk = k % 4
        load_w("wr", d_wr, k)
        t = xhp.tile([128, BC], bf16, tag=f"xh_{k}", name=f"xh_{k}")
        for half in range(2):
            c0 = BC // 2 * half
            nc.sync.dma_start(t[:, c0:c0 + BC // 2],
                              src[128 * kk:128 * (kk + 1), c0:c0 + BC // 2])
        xh[k] = t
    for k in range(KC):
        load_w("wz", d_wz, k)
    for k in range(KC):
        load_w("wh", d_wh, k)

    # rh[u] and z[u] accumulate full-width [128, 2048] results per
    # u-chunk; slices are written per (u, g) as chains retire.
    rh = [rhp.tile([128, BC], bf16, tag=f"rh_{u}", name=f"rh_{u}")
          for u in range(4)]
    zt = [zpool.tile([128, BC], bf16, tag=f"z_{u}", name=f"z_{u}")
          for u in range(4)]

    blocks = nc.m.functions[0].blocks

    def raw_last():
        return blocks[-1].instructions[-1]

    # Explicit LDWEIGHTS shared by a group of matmuls; total PE order
    # via nosync deps (LDW <- previous MM, MM <- its LDW).
    prev_mm = [None]

    def ldw_group(wname, u, k, moving_i, ps, groups, start, stop):
        """One weight load + one matmul per batch group in `groups`."""
        lhsT = w_sb[wname][:, k, 128 * u:128 * (u + 1)]
        nc.tensor.ldweights(lhsT)
        ldw = raw_last()
        if prev_mm[0] is not None:
            ldw.add_dependency(prev_mm[0], NOSYNC)
        for g in groups:
            nc.tensor.matmul(ps[g][:], lhsT,
                             moving_i[:, GW * g:GW * (g + 1)],
                             start=start, stop=stop)
            mm = raw_last()
            mm.ldweights = False
            mm.add_dependency(ldw.name, NOSYNC)
            prev_mm[0] = mm.name

    def alloc_ps(label, groups):
        return {g: psg.tile([128, GW], f32, tag="psg",
                            name=f"ps_{label}_{g}") for g in groups}

    def chain(wname, u, kslice, moving, ps=None, groups=(0, 1, 2, 3),
              first=True, last=True):
        """ps[g] += sum_k W[k][:,u].T @ moving[k][:,g] for g in groups."""
        if ps is None:
            ps = alloc_ps(f"{wname}_{u}", groups)
        nk = len(kslice)
        for i, k in enumerate(kslice):
            ldw_group(wname, u, k, moving[i], ps, groups,
                      start=(first and i == 0), stop=(last and i == nk - 1))
        return ps

    def chain_pair(wname, ua, ub, moving):
        """Two u-chains interleaved k-major: each arriving k-chunk
        unlocks 8 matmuls (all 8 PSUM banks advance one step)."""
        psa = alloc_ps(f"{wname}_{ua}", range(4))
        psb = alloc_ps(f"{wname}_{ub}", range(4))
        for i in range(KC):
            ldw_group(wname, ua, i, moving[i], psa, range(4),
                      start=(i == 0), stop=(i == KC - 1))
            ldw_group(wname, ub, i, moving[i], psb, range(4),
                      start=(i == 0), stop=(i == KC - 1))
        return psa, psb

    def evac_r(u, ps):
        for g in range(4):
            c0 = GW * g
            rT = rtp.tile([128, GW], bf16, tag="rt", name=f"rt_{u}_{g}")
            nc.scalar.activation(rT[:], ps[g][:], Sig)
            nc.vector.tensor_mul(rh[u][:, c0:c0 + GW], rT[:],
                                 xh[4 + u][:, c0:c0 + GW])

    def evac_z(u, ps):
        for g in range(4):
            nc.scalar.activation(zt[u][:, GW * g:GW * (g + 1)],
                                 ps[g][:], Sig)

    def evac_h(u, ps, groups):
        for g in groups:
            c0 = GW * g
            hh = hhp.tile([128, GW], bf16, tag="hh", name=f"hh_{u}_{g}")
            nc.scalar.activation(hh[:], ps[g][:], Tanh)
            # out = hh + z * (h_prev - hh), transposed, all-bf16 on DVE
            dt = tmpp.tile([128, GW], bf16, tag="tmp", name=f"d_{u}_{g}")
            nc.vector.tensor_sub(dt[:], xh[4 + u][:, c0:c0 + GW], hh[:])
            pt = tmpp.tile([128, GW], bf16, tag="tmp2", name=f"p_{u}_{g}")
            nc.vector.tensor_mul(pt[:], zt[u][:, c0:c0 + GW], dt[:])
            o = outp.tile([128, GW], bf16, tag="out", name=f"o_{u}_{g}")
            nc.vector.tensor_add(o[:], hh[:], pt[:])
            nc.sync.dma_start(
                d_outT[128 * u:128 * (u + 1), c0:c0 + GW], o[:])

    # ---- gate r (transposed) ----
    # Sequential u-chains: u0 is DMA-ramp-paced; u1-u3 run dense while
    # earlier chains' sigmoid evacuations free banks in the background.
    for u in range(4):
        ps = chain("wr", u, range(KC), xh)
        evac_r(u, ps)

    # ---- gate z (transposed) ----
    for u in range(4):
        ps = chain("wz", u, range(KC), xh)
        evac_z(u, ps)

    # ---- gate h (transposed) + combine + store ----
    for u in range(3):
        ps = chain("wh", u, range(4), xh, last=False)
        ps = chain("wh", u, range(4, KC), rh, ps=ps, first=False)
        evac_h(u, ps, range(4))
    # last u-chain split into two half-chains so the final tanh +
    # combine + store tail covers ~one slice instead of four
    for grp in ((0, 1), (2, 3)):
        ps = chain("wh", 3, range(4), xh, groups=grp, last=False)
        ps = chain("wh", 3, range(4, KC), rh, ps=ps, groups=grp,
                   first=False)
        evac_h(3, ps, grp)

    est.close()


_NC_CACHE = {}


def _build():
    if "nc" in _NC_CACHE:
        return _NC_CACHE["nc"]
    import concourse.tile as tile
    from concourse import bacc, mybir

    bf16 = mybir.dt.bfloat16
    nc = bacc.Bacc("TRN2", target_bir_lowering=False, debug=False)
    d_xT = nc.dram_tensor("xT", [D, BC], bf16, kind="ExternalInput").ap()
    d_hT = nc.dram_tensor("hT", [U, BC], bf16, kind="ExternalInput").ap()
    d_wz = nc.dram_tensor("Wz", [K, U], bf16, kind="ExternalInput").ap()
    d_wr = nc.dram_tensor("Wr", [K, U], bf16, kind="ExternalInput").ap()
    d_wh = nc.dram_tensor("Wh", [K, U], bf16, kind="ExternalInput").ap()
    d_outT = nc.dram_tensor("outT", [U, BC], bf16, kind="ExternalOutput").ap()

    with tile.TileContext(nc) as tc:
        build_gru_tile_kernel(tc, d_xT, d_hT, d_wz, d_wr, d_wh, d_outT)
    nc.compile()
    _NC_CACHE["nc"] = nc
    return nc


def run_sharded(inputs, h_prev, Wz, Wr, Wh, trace=False):
    import ml_dtypes
    from concourse.bass_utils import run_bass_kernel_spmd

    bf16 = ml_dtypes.bfloat16
    nc = _build()
    inputs = np.asarray(inputs, dtype=np.float32).astype(bf16)
    h_prev = np.asarray(h_prev, dtype=np.float32).astype(bf16)
    Wzq = np.ascontiguousarray(np.asarray(Wz, dtype=np.float32).astype(bf16))
    Wrq = np.ascontiguousarray(np.asarray(Wr, dtype=np.float32).astype(bf16))
    Whq = np.ascontiguousarray(np.asarray(Wh, dtype=np.float32).astype(bf16))
    in_maps = []
    for i in range(N_CORES):
        in_maps.append({
            "xT": np.ascontiguousarray(inputs[i * BC:(i + 1) * BC].T),
            "hT": np.ascontiguousarray(h_prev[i * BC:(i + 1) * BC].T),
            "Wz": Wzq,
            "Wr": Wrq,
            "Wh": Whq,
        })
    res = run_bass_kernel_spmd(
        nc, in_maps, core_ids=list(range(N_CORES)), trace=trace
    )
    out = np.concatenate(
        [np.asarray(res.results[i]["outT"]).T for i in range(N_CORES)], axis=0
    ).astype(np.float32)
    return out, res


def kernel(inputs, h_prev, Wz, Wr, Wh):
    out, _ = run_sharded(inputs, h_prev, Wz, Wr, Wh, trace=False)
    return out
